# revision 11
# baseline (speedup 1.0000x reference)
"""Trainium2 Bass kernel for nn_DilatedAttention (B=2, L=4096, E=512, H=8, D=64,
dilation=2, window=256, causal, pre-norm transformer block with MLP).

Strategy
--------
* 8 cores, sequence-parallel: core c owns tokens [512c, 512c+512) of both
  batches.  The attention mask is local (|i-j| <= 256), so each core also
  computes K/V for a 256-token halo before its range (zero-padded for c=0).
* The dilation-2 + causal + window mask couples only equal-parity positions:
  even tokens attend even tokens, odd attend odd.  After de-interleaving by
  parity the mask is a plain causal sliding-window (window=128) attention over
  a length-2048 subsequence.  Parity de-interleave is free: activations are
  kept feature-major ([feature, token]) so parity is a stride-2 slice on the
  free axis.
* Feature-major layout everywhere: weights are used directly as matmul lhsT
  (stationary) operands, activations as the moving operand.  LayerNorm is
  decomposed as h = x*rstd + z with z = -mu*rstd (per-token rows broadcast
  across partitions via gpsimd.partition_broadcast); the gain/bias are folded
  into the weights on the host.
* Matmul inputs in bf16 (fp32 PSUM accumulation); the residual stream stays
  fp32.  LN statistics use fp32r (full-rate fp32) matmuls against a ones
  column.
"""

import os
import numpy as np
import ml_dtypes

import concourse.bass as bass
import concourse.mybir as mybir
import concourse.tile as tile
from concourse import library_config
from concourse.bass_utils import run_bass_kernel_spmd
from concourse.masks import make_identity

F32 = mybir.dt.float32
F32R = mybir.dt.float32r
BF16 = mybir.dt.bfloat16
AF = mybir.ActivationFunctionType
ALU = mybir.AluOpType

# problem constants
B, L, E, H, D = 2, 4096, 512, 8, 64
HID = 2048
EPS = 1e-5
WIN, DIL = 256, 2
N_CORES = 8
S = L // N_CORES          # tokens per core per batch (512)
HALO = WIN                # kv halo tokens (256)
T_EXT = S + HALO          # 768
EC = E // 128             # 4 feature chunks
HC = HID // 128           # 16 hidden chunks
NQ = S // 2               # queries per parity (256)
KB = (NQ + 128) // 128    # key blocks per parity (3)
QB = NQ // 128            # query blocks per parity (2)
MASK_VAL = -10000.0


def _legalize_waits(m, max_waits=1):
    """The walrus build here accepts only one sync-wait command per lowered
    instruction; hoist extras onto same-engine NoOps placed just before."""
    for fn in m.functions:
        for blk in fn.blocks:
            new_list = []
            for ins in blk.instructions:
                si = ins.sync_info
                if si is not None and si.on_wait is not None and len(si.on_wait) > max_waits:
                    waits = list(si.on_wait)
                    extra, keep = waits[:-max_waits], waits[-max_waits:]
                    k = 0
                    while extra:
                        chunk, extra = extra[:max_waits], extra[max_waits:]
                        nop = mybir.InstNoOp(name=f"{ins.name}-wsplit{k}", ins=[], outs=[])
                        nop.engine = ins.engine
                        nop.sync_info = mybir.SyncInfo(on_wait=chunk, on_update=[])
                        new_list.append(nop)
                        k += 1
                    si.on_wait = keep
                new_list.append(ins)
            blk.instructions = new_list


def build_program(has_qk_bias: bool, has_v_bias: bool, has_out_bias: bool, has_b2: bool):
    nc = bass.Bass("TRN2", target_bir_lowering=False, debug=False)

    # ---- DRAM I/O ----
    xT = nc.dram_tensor("xT", [B, E, T_EXT], F32, kind="ExternalInput").ap()
    wqkv = nc.dram_tensor("wqkv", [E, 3 * E], BF16, kind="ExternalInput").ap()
    wout = nc.dram_tensor("wout", [E, E], BF16, kind="ExternalInput").ap()
    w1 = nc.dram_tensor("w1", [E, HID], BF16, kind="ExternalInput").ap()
    w2 = nc.dram_tensor("w2", [HID, E], BF16, kind="ExternalInput").ap()
    mask_in = nc.dram_tensor("mask", [KB, 128, NQ], F32, kind="ExternalInput").ap()
    vmlp_in = nc.dram_tensor("vmlp", [HID], F32, kind="ExternalInput").ap()
    urow_in = nc.dram_tensor("urow", [3 * E], BF16, kind="ExternalInput").ap()
    u2row_in = nc.dram_tensor("u2row", [HID], BF16, kind="ExternalInput").ap()
    if has_qk_bias:
        vqk_in = nc.dram_tensor("vqk", [2 * E], F32, kind="ExternalInput").ap()
    if has_v_bias:
        vvb_in = nc.dram_tensor("vvb", [E], F32, kind="ExternalInput").ap()
    if has_out_bias:
        outb_in = nc.dram_tensor("outb", [E], F32, kind="ExternalInput").ap()
    if has_b2:
        b2_in = nc.dram_tensor("b2v", [E], F32, kind="ExternalInput").ap()
    yT = nc.dram_tensor("yT", [B, E, S], F32, kind="ExternalOutput").ap()

    with tile.TileContext(nc) as tc:
        ctxstack = []

        def pool(name, bufs, space="SBUF"):
            p = tc.tile_pool(name=name, bufs=bufs, space=space)
            ctxstack.append(p)
            return p.__enter__()

        wpool = pool("wpool", 1)
        xpool = pool("xpool", 1)
        x1pool = pool("x1pool", 1)
        qkpool = pool("qkpool", 1)
        vpool = pool("vpool", 1)
        ptpool = pool("ptpool", 3)
        scpool = pool("scpool", 4)
        opool = pool("opool", 1)
        x2pool = pool("x2pool", 1)
        hpool = pool("hpool", 1)
        ypool = pool("ypool", 2)
        stpool = pool("stpool", 2)
        bcpool = pool("bcpool", 2)
        sqpool = pool("sqpool", 3)
        rpool = pool("rpool", 4)

        pstat = pool("pstat", 2, space="PSUM")
        psbp = pool("psbp", 1, space="PSUM")
        pmain = pool("pmain", 3, space="PSUM")
        patt = pool("patt", 2, space="PSUM")

        # ---- constants / weights into SBUF ----
        wqkv_sb = wpool.tile([128, EC, 3 * E], BF16)
        nc.sync.dma_start(wqkv_sb, wqkv.rearrange("(c p) f -> p c f", p=128))
        wout_sb = wpool.tile([128, EC, E], BF16)
        nc.sync.dma_start(wout_sb, wout.rearrange("(c p) f -> p c f", p=128))
        w1_sb = wpool.tile([128, EC, HID], BF16)
        nc.sync.dma_start(w1_sb, w1.rearrange("(c p) f -> p c f", p=128))
        w2_sb = wpool.tile([128, HC, E], BF16)
        nc.sync.dma_start(w2_sb, w2.rearrange("(c p) f -> p c f", p=128))
        mask_sb = wpool.tile([128, KB, NQ], F32)
        nc.sync.dma_start(mask_sb, mask_in.rearrange("k p q -> p k q"))
        vmlp_sb = wpool.tile([128, HC], F32)
        nc.sync.dma_start(vmlp_sb, vmlp_in.rearrange("(s p) -> p s", p=128))
        urow_sb = wpool.tile([1, 3 * E], BF16)
        nc.sync.dma_start(urow_sb, urow_in[None, :])
        u2row_sb = wpool.tile([1, HID], BF16)
        nc.sync.dma_start(u2row_sb, u2row_in[None, :])
        ones_row = wpool.tile([1, 128], BF16)
        nc.vector.memset(ones_row, 1.0)
        if has_qk_bias:
            vqk_sb = wpool.tile([128, 8], F32)
            nc.sync.dma_start(vqk_sb, vqk_in.rearrange("(s p) -> p s", p=128))
        if has_v_bias:
            vvb_sb = wpool.tile([128, E], F32)
            nc.sync.dma_start(vvb_sb, vvb_in[None, :].to_broadcast([128, E]))
        if has_out_bias:
            outb_sb = wpool.tile([128, EC], F32)
            nc.sync.dma_start(outb_sb, outb_in.rearrange("(s p) -> p s", p=128))
        if has_b2:
            b2_sb = wpool.tile([128, EC], F32)
            nc.sync.dma_start(b2_sb, b2_in.rearrange("(s p) -> p s", p=128))

        ident = wpool.tile([128, 128], BF16)
        make_identity(nc, ident)
        ones_col = wpool.tile([128, 1], BF16)
        nc.vector.memset(ones_col, 1.0)
        eps_t = wpool.tile([1, 1], F32)
        nc.vector.memset(eps_t, EPS)

        def layernorm_stats(xt_ap, T, bc_w, tag):
            """xt_ap: [128, EC, T] fp32 feature-major slab.
            Returns (rstd_b [128,T] f32, z_b [128,T] bf16) broadcast tiles."""
            n_tt = (T + 383) // 384
            tts = [(i * T // n_tt, (i + 1) * T // n_tt) for i in range(n_tt)]
            mu_full = stpool.tile([1, T_EXT], F32, tag="mu", name="mu")
            tmp_full = stpool.tile([1, T_EXT], F32, tag="sttmp", name="sttmp")
            rstd_full = stpool.tile([1, T_EXT], F32, tag="strstd", name="strstd")
            mu_neg, tmp, rstd = mu_full[:, :T], tmp_full[:, :T], rstd_full[:, :T]
            # bf16 shadow of x for the ones-matmul stats (error ~1e-4 on mu/rstd)
            xbf_full = sqpool.tile([128, EC, T_EXT], BF16, tag="xbf", name="xbf")
            xbf = xbf_full[:, :, :T]
            for c in range(EC):
                nc.gpsimd.tensor_copy(xbf[:, c, :], xt_ap[:, c, :])
            for (t0, t1) in tts:
                ps_s = pstat.tile([1, t1 - t0], F32, tag="pstat")
                for c in range(EC):
                    nc.tensor.matmul(ps_s, lhsT=ones_col, rhs=xbf[:, c, t0:t1],
                                     start=(c == 0), stop=(c == EC - 1))
                nc.scalar.mul(mu_neg[:, t0:t1], ps_s, -1.0 / E)
                ps_q = pstat.tile([1, t1 - t0], F32, tag="pstat")
                for c in range(EC):
                    xsq_full = sqpool.tile([128, 512], BF16, tag="xsq", name="xsq")
                    xsq = xsq_full[:, : t1 - t0]
                    nc.scalar.square(xsq, xt_ap[:, c, t0:t1])
                    nc.tensor.matmul(ps_q, lhsT=ones_col, rhs=xsq,
                                     start=(c == 0), stop=(c == EC - 1))
                nc.scalar.mul(tmp[:, t0:t1], ps_q, 1.0 / E)
            # var = E[x^2] - mu^2 ; rstd = 1/sqrt(var+eps) ; z = -mu*rstd
            musq = stpool.tile([1, T_EXT], F32, tag="stmusq", name="stmusq")[:, :T]
            nc.scalar.square(musq, mu_neg)
            nc.vector.tensor_tensor(tmp, tmp, musq, ALU.subtract)
            nc.scalar.activation(tmp, tmp, AF.Sqrt, bias=eps_t)
            nc.vector.reciprocal(rstd, tmp)
            zrow = stpool.tile([1, T_EXT], BF16, tag="stz", name="stz")[:, :T]
            rstd_bf = stpool.tile([1, T_EXT], BF16, tag="strbf", name="strbf")[:, :T]
            nc.vector.tensor_copy(rstd_bf, rstd)
            nc.vector.tensor_tensor(zrow, mu_neg, rstd_bf, ALU.mult)
            return rstd_bf, zrow

        for b in range(B):
            # ---- stage A: load x, LN1 stats, normalize ----
            xt = xpool.tile([128, EC, T_EXT], F32, tag="xt")
            nc.sync.dma_start(xt, xT[b].rearrange("(c p) t -> p c t", p=128))
            rstd_bf, zrow = layernorm_stats(xt, T_EXT, bcpool, "ln1")
            x1 = x1pool.tile([128, EC, T_EXT], BF16, tag="x1")
            for tt in range(2):
                t0, t1 = tt * 384, (tt + 1) * 384
                ps_b = psbp.tile([128, 384], F32, tag="psb")
                nc.tensor.matmul(ps_b, lhsT=ones_row, rhs=rstd_bf[:, t0:t1],
                                 start=True, stop=True)
                for c in range(EC):
                    nc.vector.tensor_tensor(x1[:, c, t0:t1], xt[:, c, t0:t1], ps_b, ALU.mult)
            x1_par = x1.rearrange("p c (t two) -> p c two t", two=2)
            zrow_par = zrow.rearrange("o (t two) -> o two t", two=2)

            # ---- stage B: QKV ----
            # Q,K feature-major [f, t]
            qkT = qkpool.tile([128, 8, T_EXT], BF16, tag="qkT")
            for fs in range(8):
                for tt in range(2):
                    t0, t1 = tt * 384, (tt + 1) * 384
                    ps = pmain.tile([128, 384], F32, tag="pmain")
                    for c in range(EC):
                        nc.tensor.matmul(ps, lhsT=wqkv_sb[:, c, fs * 128:(fs + 1) * 128],
                                         rhs=x1[:, c, t0:t1],
                                         start=(c == 0), stop=False)
                    nc.tensor.matmul(ps, lhsT=urow_sb[:, fs * 128:(fs + 1) * 128],
                                     rhs=zrow[:, t0:t1], start=False, stop=True)
                    if has_qk_bias:
                        nc.vector.tensor_scalar(qkT[:, fs, t0:t1], ps,
                                                vqk_sb[:, fs:fs + 1], None, ALU.add)
                    else:
                        nc.vector.tensor_copy(qkT[:, fs, t0:t1], ps)
            qkT_par = qkT.rearrange("p s (t two) -> p s two t", two=2)

            # V token-major, parity-separated, with a ones column appended per head
            vplus = [vpool.tile([128, KB, H, D + 1], BF16, tag=f"vplus{p}", name=f"vplus{p}") for p in range(2)]
            for par in range(2):
                nc.vector.memset(vplus[par][:, :, :, D:D + 1], 1.0)
                for kb in range(KB):
                    ps = pmain.tile([128, E], F32, tag="pmain")
                    for c in range(EC):
                        nc.tensor.matmul(ps, lhsT=x1_par[:, c, par, kb * 128:(kb + 1) * 128],
                                         rhs=wqkv_sb[:, c, 2 * E:3 * E],
                                         start=(c == 0), stop=False)
                    nc.tensor.matmul(ps, lhsT=zrow_par[:, par, kb * 128:(kb + 1) * 128],
                                     rhs=urow_sb[:, 2 * E:3 * E], start=False, stop=True)
                    pv = ps.rearrange("p (h d) -> p h d", h=H)
                    if has_v_bias:
                        nc.vector.tensor_tensor(vplus[par][:, kb, :, 0:D], pv,
                                                vvb_sb.rearrange("p (h d) -> p h d", h=H),
                                                ALU.add)
                    else:
                        nc.vector.tensor_copy(vplus[par][:, kb, :, 0:D], pv)

            # ---- stage C: attention ----
            oslab = [opool.tile([128, QB, E], BF16, tag=f"oslab{p}", name=f"oslab{p}") for p in range(2)]
            for par in range(2):
                for h in range(H):
                    rb = (h % 2) * 64
                    sl = h // 2
                    pt = ptpool.tile([128, KB, NQ], BF16, tag="pt")
                    for kb in range(KB):
                        ps_sc = patt.tile([128, NQ], F32, tag="patt")
                        nc.tensor.matmul(
                            ps_sc,
                            lhsT=qkT_par[rb:rb + 64, 4 + sl, par, kb * 128:(kb + 1) * 128],
                            rhs=qkT_par[rb:rb + 64, sl, par, 128:128 + NQ],
                            start=True, stop=True)
                        sc = scpool.tile([128, NQ], BF16, tag="sc")
                        nc.vector.tensor_tensor(sc, ps_sc, mask_sb[:, kb, :], ALU.add)
                        nc.scalar.activation(pt[:, kb, :], sc, AF.Exp)
                    for qb in range(QB):
                        ps_o = patt.tile([128, D + 1], F32, tag="patt")
                        for kb in range(KB):
                            nc.tensor.matmul(ps_o, lhsT=pt[:, kb, qb * 128:(qb + 1) * 128],
                                             rhs=vplus[par][:, kb, h, :],
                                             start=(kb == 0), stop=(kb == KB - 1))
                        rin = rpool.tile([128, 1], F32, tag="rin")
                        nc.vector.reciprocal(rin, ps_o[:, D:D + 1])
                        nc.vector.tensor_scalar(oslab[par][:, qb, h * D:(h + 1) * D],
                                                ps_o[:, 0:D], rin, None, ALU.mult)
            # transpose O to feature-major, interleaving parities back
            otT = opool.tile([128, EC, S], BF16, tag="otT")
            otT_par = otT.rearrange("p c (t two) -> p c two t", two=2)
            for par in range(2):
                for qb in range(QB):
                    for fc in range(EC):
                        ps_t = patt.tile([128, 128], BF16, tag="patt")
                        nc.tensor.transpose(ps_t, oslab[par][:, qb, fc * 128:(fc + 1) * 128], ident)
                        nc.vector.tensor_copy(otT_par[:, fc, par, qb * 128:(qb + 1) * 128], ps_t)

            # ---- stage D: out-proj + residual ----
            x2T = x2pool.tile([128, EC, S], F32, tag="x2T")
            for es in range(EC):
                ps = pmain.tile([128, S], F32, tag="pmain")
                for c in range(EC):
                    nc.tensor.matmul(ps, lhsT=wout_sb[:, c, es * 128:(es + 1) * 128],
                                     rhs=otT[:, c, :], start=(c == 0), stop=(c == EC - 1))
                if has_out_bias:
                    nc.vector.tensor_scalar(ps, ps, outb_sb[:, es:es + 1], None, ALU.add)
                nc.vector.tensor_tensor(x2T[:, es, :], ps, xt[:, es, HALO:T_EXT], ALU.add)

            # ---- stage E: LN2 ----
            rstd2_bf, z2row = layernorm_stats(x2T, S, bcpool, "ln2")
            x21 = x2pool.tile([128, EC, S], BF16, tag="x21")
            ps_b2 = psbp.tile([128, S], F32, tag="psb")
            nc.tensor.matmul(ps_b2, lhsT=ones_row, rhs=rstd2_bf, start=True, stop=True)
            for c in range(EC):
                nc.vector.tensor_tensor(x21[:, c, :], x2T[:, c, :], ps_b2, ALU.mult)

            # ---- stage F: MLP ----
            h2T = hpool.tile([128, HC, S], BF16, tag="h2T")
            for hs in range(HC):
                ps = pmain.tile([128, S], F32, tag="pmain")
                for c in range(EC):
                    nc.tensor.matmul(ps, lhsT=w1_sb[:, c, hs * 128:(hs + 1) * 128],
                                     rhs=x21[:, c, :], start=(c == 0), stop=False)
                nc.tensor.matmul(ps, lhsT=u2row_sb[:, hs * 128:(hs + 1) * 128],
                                 rhs=z2row, start=False, stop=True)
                nc.scalar.activation(h2T[:, hs, :], ps, AF.Gelu, bias=vmlp_sb[:, hs:hs + 1])
            for es in range(EC):
                ps = pmain.tile([128, S], F32, tag="pmain")
                for hc in range(HC):
                    nc.tensor.matmul(ps, lhsT=w2_sb[:, hc, es * 128:(es + 1) * 128],
                                     rhs=h2T[:, hc, :], start=(hc == 0), stop=(hc == HC - 1))
                if has_b2:
                    nc.vector.tensor_scalar(ps, ps, b2_sb[:, es:es + 1], None, ALU.add)
                yt = ypool.tile([128, S], F32, tag="yt")
                nc.vector.tensor_tensor(yt, ps, x2T[:, es, :], ALU.add)
                nc.sync.dma_start(yT[b, es * 128:(es + 1) * 128, :], yt)

        for p in reversed(ctxstack):
            p.__exit__(None, None, None)

    return nc


_cached = {}


def _get_program(key):
    if key not in _cached:
        nc = build_program(*key)
        _legalize_waits(nc.m)
        _cached[key] = nc
    return _cached[key]


def _prepare_core_inputs(inputs):
    """Host-side folding + sharding. Returns (flags_key, in_maps list)."""
    x = np.asarray(inputs["x"], np.float32)
    ln1_g = np.asarray(inputs["ln1_g"], np.float32)
    ln1_b = np.asarray(inputs["ln1_b"], np.float32)
    qkv_w = np.asarray(inputs["qkv_w"], np.float32)
    qkv_b = np.asarray(inputs["qkv_b"], np.float32)
    out_w = np.asarray(inputs["out_w"], np.float32)
    out_b = np.asarray(inputs["out_b"], np.float32)
    ln2_g = np.asarray(inputs["ln2_g"], np.float32)
    ln2_b = np.asarray(inputs["ln2_b"], np.float32)
    w1 = np.asarray(inputs["w1"], np.float32)
    b1 = np.asarray(inputs["b1"], np.float32)
    w2 = np.asarray(inputs["w2"], np.float32)
    b2 = np.asarray(inputs["b2"], np.float32)

    # fold LN1 gain into qkv_w; fold attention 1/sqrt(D) into the Q part
    qscale = 1.0 / np.sqrt(D)
    wqkv_eff = ln1_g[:, None] * qkv_w
    vqkv = ln1_b @ qkv_w + qkv_b          # [3E]
    wqkv_eff[:, :E] *= qscale
    vqkv = vqkv.copy()
    vqkv[:E] *= qscale
    # fold LN2 gain into w1
    w1_eff = ln2_g[:, None] * w1
    vmlp = ln2_b @ w1 + b1                # [HID]

    has_qk_bias = bool(np.any(vqkv[: 2 * E] != 0.0))
    has_v_bias = bool(np.any(vqkv[2 * E:] != 0.0))
    has_out_bias = bool(np.any(out_b != 0.0))
    has_b2 = bool(np.any(b2 != 0.0))
    key = (has_qk_bias, has_v_bias, has_out_bias, has_b2)

    wqkv_bf = wqkv_eff.astype(ml_dtypes.bfloat16)
    wout_bf = out_w.astype(ml_dtypes.bfloat16)
    w1_bf = w1_eff.astype(ml_dtypes.bfloat16)
    w2_bf = w2.astype(ml_dtypes.bfloat16)
    urow_bf = wqkv_bf.astype(np.float32).sum(0).astype(ml_dtypes.bfloat16)
    u2row_bf = w1_bf.astype(np.float32).sum(0).astype(ml_dtypes.bfloat16)

    # x transposed per batch with halo: [B, E, T_EXT]
    xT_full = np.ascontiguousarray(x.transpose(0, 2, 1))  # [B, E, L]
    in_maps = []
    kk = np.arange(KB * 128)[:, None]     # key index within the window strip
    qq = np.arange(NQ)[None, :]
    base_valid = (qq <= kk) & (kk <= qq + 128)
    for c in range(N_CORES):
        s = c * S
        xTe = np.zeros((B, E, T_EXT), np.float32)
        lo = s - HALO
        src_lo = max(lo, 0)
        xTe[:, :, src_lo - lo:] = xT_full[:, :, src_lo:s + S]
        valid = base_valid if c > 0 else (base_valid & (kk >= 128))
        mask = np.where(valid, 0.0, MASK_VAL).astype(np.float32)
        mask = mask.reshape(KB, 128, NQ)
        im = {
            "xT": xTe,
            "wqkv": wqkv_bf,
            "wout": wout_bf,
            "w1": w1_bf,
            "w2": w2_bf,
            "mask": mask,
            "vmlp": vmlp.astype(np.float32),
            "urow": urow_bf,
            "u2row": u2row_bf,
        }
        if has_qk_bias:
            im["vqk"] = vqkv[: 2 * E].astype(np.float32)
        if has_v_bias:
            im["vvb"] = vqkv[2 * E:].astype(np.float32)
        if has_out_bias:
            im["outb"] = out_b.astype(np.float32)
        if has_b2:
            im["b2v"] = b2.astype(np.float32)
        in_maps.append(im)
    return key, in_maps


_last_results = None


def kernel(**inputs) -> np.ndarray:
    global _last_results
    key, in_maps = _prepare_core_inputs(inputs)
    nc = _get_program(key)
    res = run_bass_kernel_spmd(nc, in_maps, core_ids=list(range(N_CORES)))
    _last_results = res
    out = np.empty((B, L, E), np.float32)
    for c in range(N_CORES):
        yT = res.results[c]["yT"]          # [B, E, S]
        out[:, c * S:(c + 1) * S, :] = yT.transpose(0, 2, 1)
    return out
